# revision 10
# baseline (speedup 1.0000x reference)
"""Trainium2 Bass kernel for nn_MoE (moe_routing) — expert dispatch.

Strategy: true top-2 expert parallelism across 8 NeuronCores. The host
computes the gating network (softmax over x @ gate_w.T, 0.05% of total
FLOPs) and the top-2 routing decision, then *dispatches*: for expert e it
gathers the ~1071 tokens routed to e, pads to the shared capacity C
(= max expert load), and sends them to core e along with that core's
per-token gate values. Core e runs its expert FFN densely over its C
tokens:

    yT_e = (gelu(x_e @ w1[e] + b1[e]) @ w2[e] + b2[e]) * g_e

entirely on device (both GEMMs on the PE in bf16 with fp32 PSUM
accumulation, gelu+b1 on the scalar engine, (y+b2)*g fused into a single
vector op). The host then *combines*: y[idx_e] += yT_e, exact because
each token appears in exactly its two routed experts' lists.

Compared to the dense-per-expert formulation (every core computes all
4096 tokens) this does 4096/C ≈ 3.8x less PE work per core.

mm2 is laid out output-stationary over d_model chunks ([d_out partition,
token free]) so b2 becomes a per-partition bias and no extra rank-1
matmul is needed.

MOE_DT=f32r forces float32r matmuls (capacity rounded to 256, ~2x the
weight DMA, higher precision). MOE_REPS>1 repeats the sweep (timing).
"""

import os
from contextlib import ExitStack

import numpy as np
import ml_dtypes

import concourse.bass as bass
from concourse import bacc
import concourse.mybir as mybir
import concourse.tile as tile
from concourse.bass_utils import run_bass_kernel_spmd

F32 = mybir.dt.float32
BF16 = mybir.dt.bfloat16
F32R = mybir.dt.float32r
AF = mybir.ActivationFunctionType
ALU = mybir.AluOpType

D_MODEL = 1024
D_HEAD = 2048
N_EXPERTS = 8
TOP_K = 2
N_CORES = 8

DC = D_MODEL // 128      # d_model chunks of 128 (mm1 contraction steps)
HC = D_HEAD // 128       # d_head chunks of 128
DOC = D_MODEL // 128     # d_model output chunks (mm2 output partitions)
NG1 = 4                  # w1 DMA groups (d_head split in 4, for early start)

LAST_RESULT = None       # BassKernelResults of the most recent run (for test.py)


def _cfg():
    if os.environ.get("MOE_DT") == "f32r":
        return F32R, np.float32, 256
    return BF16, ml_dtypes.bfloat16, 128


def plan_blocks(max_load):
    """Token blocks per core: TB-sized plus a merged/own remainder block.

    TB=512 fills a whole PSUM bank (512 f32 = 2KB/partition) and halves
    the matmul instruction count vs 256."""
    mmdt, _, quant = _cfg()
    TB = int(os.environ.get("MOE_TB", "512" if mmdt == BF16 else "256"))
    C = ((max(max_load, 1) + quant - 1) // quant) * quant if quant == 256 else max(
        max_load, 1
    )
    n_full, rem = divmod(C, TB)
    if rem == 0:
        blks = [TB] * n_full
    elif TB + rem <= 512 and n_full >= 1:
        blks = [TB] * (n_full - 1) + [TB + rem]
    else:
        blks = [TB] * n_full + [rem]
    return C, blks


def build_nc(C, blks):
    """Build the single-core SPMD Bass program (expert FFN over C tokens)."""
    mmdt, _, _ = _cfg()
    reps = int(os.environ.get("MOE_REPS", "1"))
    hw_loop = os.environ.get("MOE_HW_LOOP") == "1"
    pe_only = os.environ.get("MOE_PE_ONLY") == "1"  # timing probe: matmuls only
    max_tb = max(blks)
    nc = bacc.Bacc()

    # x laid out block-major so each block's DMA is one contiguous stretch:
    # xflat[p, DC*t0 + c*tb + t] = x_e[t0+t, c*128+p]
    x_d = nc.declare_dram_parameter("xf", [128, DC * C], mmdt, isOutput=False)
    w1_d = nc.declare_dram_parameter("w1g", [128, NG1, DC, D_HEAD // NG1], mmdt,
                                     isOutput=False)
    w2_d = nc.declare_dram_parameter("w2g", [128, DOC, HC, 128], mmdt,
                                     isOutput=False)
    b1t_d = nc.declare_dram_parameter("b1t", [128, HC], F32, isOutput=False)
    b2t_d = nc.declare_dram_parameter("b2t", [128, DOC], F32, isOutput=False)
    g_d = nc.declare_dram_parameter("g", [128, C], F32, isOutput=False)
    out_d = nc.declare_dram_parameter("out", [D_MODEL, C], F32, isOutput=True)

    with tile.TileContext(nc) as tc, ExitStack() as ctx:
        psb = int(os.environ.get("MOE_PSB", "4"))
        singles = ctx.enter_context(tc.tile_pool(name="singles", bufs=1))
        ht_pool = ctx.enter_context(tc.tile_pool(name="ht", bufs=3))
        y_pool = ctx.enter_context(tc.tile_pool(name="yb", bufs=4))
        ps_h = ctx.enter_context(tc.tile_pool(name="ps_h", bufs=psb, space="PSUM"))
        ps_y = ctx.enter_context(tc.tile_pool(name="ps_y", bufs=psb, space="PSUM"))

        # w1 as 4 independent d_head-group tiles so block 0's mm1 can begin
        # after ~1/4 of w1 has landed. Weight traffic rides the gpsimd
        # queues so x/g/out DMAs on nc.sync are never stuck behind it.
        HG = D_HEAD // NG1
        w1_sb = []
        for gidx in range(NG1):
            t = singles.tile([128, DC, HG], mmdt, name=f"w1g{gidx}")
            w1_sb.append(t)
            nc.gpsimd.dma_start(out=t, in_=w1_d[:, gidx])
        w2_sb = singles.tile([128, DOC, HC, 128], mmdt)
        nc.gpsimd.dma_start(out=w2_sb, in_=w2_d[:])
        b1t_sb = singles.tile([128, HC], F32)
        b2t_sb = singles.tile([128, DOC], F32)
        g_sb = singles.tile([128, C], F32)
        nc.sync.dma_start(out=b1t_sb, in_=b1t_d[:])
        nc.sync.dma_start(out=b2t_sb, in_=b2t_d[:])
        nc.sync.dma_start(out=g_sb, in_=g_d[:])

        # per-block x tiles (loaded once, reused across reps)
        x_sb = []
        t0 = 0
        for tb in blks:
            t = singles.tile([128, DC, tb], mmdt, name=f"x{t0}")
            x_sb.append(t)
            nc.sync.dma_start(out=t, in_=x_d[:, DC * t0 : DC * (t0 + tb)])
            t0 += tb

        hT_const = None
        if pe_only:
            hT_const = singles.tile([128, HC, max_tb], mmdt, name="hTc")
            pc = ps_h.tile([128, max_tb], F32, tag="ph")
            tb0 = blks[0]
            nc.tensor.matmul(
                pc[:, :tb0], lhsT=w2_sb[:, 0, 0], rhs=x_sb[0][:, 0],
                start=True, stop=True,
            )
            for hc in range(HC):
                nc.scalar.activation(hT_const[:, hc, :tb0], pc[:, :tb0], AF.Gelu)
                if max_tb > tb0:
                    nc.scalar.activation(
                        hT_const[:, hc, tb0:], pc[:, : max_tb - tb0], AF.Gelu
                    )

        def emit_mm1(blk):
            tb = blks[blk]
            hT = ht_pool.tile([128, HC, max_tb], mmdt, tag="hT")
            for hc in range(HC):
                ph = ps_h.tile([128, max_tb], F32, tag="ph")
                w1t = w1_sb[hc // (HC // NG1)]
                hcl = hc % (HC // NG1)
                for dc in range(DC):
                    nc.tensor.matmul(
                        ph[:, :tb],
                        lhsT=w1t[:, dc, hcl * 128 : (hcl + 1) * 128],
                        rhs=x_sb[blk][:, dc],
                        start=(dc == 0),
                        stop=(dc == DC - 1),
                    )
                if pe_only:
                    nc.vector.tensor_scalar_mul(
                        hT[:, hc, 0:1], ph[:, 0:1], b1t_sb[:, hc : hc + 1]
                    )
                else:
                    nc.scalar.activation(
                        hT[:, hc, :tb], ph[:, :tb], AF.Gelu,
                        bias=b1t_sb[:, hc : hc + 1],
                    )
            return hT_const if pe_only else hT

        def emit_mm2(blk, hT):
            tb = blks[blk]
            t0 = sum(blks[:blk])
            for j in range(DOC):
                py = ps_y.tile([128, max_tb], F32, tag="py")
                for hc in range(HC):
                    nc.tensor.matmul(
                        py[:, :tb],
                        lhsT=w2_sb[:, j, hc],
                        rhs=hT[:, hc, :tb],
                        start=(hc == 0),
                        stop=(hc == HC - 1),
                    )
                y_sb = y_pool.tile([128, max_tb], F32, tag="y_sb")
                # y = (mm2 + b2[j-chunk]) * gate   in one vector op
                nc.vector.scalar_tensor_tensor(
                    y_sb[:, :tb], py[:, :tb], b2t_sb[:, j : j + 1],
                    g_sb[:, t0 : t0 + tb],
                    op0=ALU.add, op1=ALU.mult,
                )
                nc.sync.dma_start(
                    out=out_d[j * 128 : (j + 1) * 128, t0 : t0 + tb],
                    in_=y_sb[:, :tb],
                )

        # Software pipeline: emit mm1(b+1) before mm2(b) so the PE streams
        # mm1(b+1) while block b's last gelu chunks drain on the scalar
        # engine — no PE bubble waiting for hT(b).
        nb = len(blks)

        def emit_sweep():
            hT_prev = None
            for blk in range(nb):
                hT_b = emit_mm1(blk)
                if hT_prev is not None:
                    emit_mm2(blk - 1, hT_prev)
                hT_prev = hT_b
            emit_mm2(nb - 1, hT_prev)

        if reps == 1:
            emit_sweep()
        elif hw_loop:
            # hardware loop: per-iteration barrier breaks the cross-sweep
            # pipeline, overestimating one sweep
            with tc.For_i(0, reps):
                emit_sweep()
        else:
            for _ in range(reps):
                emit_sweep()

    return nc


def route(x2d, gate_w):
    """Host gating: fp32 softmax + top-2, mirroring the reference."""
    logits = x2d @ gate_w.T
    m = logits.max(-1, keepdims=True)
    p = np.exp(logits - m)
    p /= p.sum(-1, keepdims=True)
    top2 = np.argsort(-p, axis=1, kind="stable")[:, :TOP_K]
    idx, gates = [], []
    for e in range(N_EXPERTS):
        ie = np.where((top2[:, 0] == e) | (top2[:, 1] == e))[0]
        idx.append(ie)
        gates.append(p[ie, e].astype(np.float32))
    return idx, gates


def make_in_maps(x2d, idx, gates, w1, b1, w2, b2, C, blks):
    _, npdt, _ = _cfg()
    in_maps = []
    for e in range(N_CORES):
        ie = idx[e]
        xe = np.zeros((C, D_MODEL), np.float32)
        xe[: len(ie)] = x2d[ie]
        xe = xe.astype(npdt)
        # block-major contiguous layout: [128, sum_b DC*tb]
        segs = []
        t0 = 0
        for tb in blks:
            seg = xe[t0 : t0 + tb].T.reshape(DC, 128, tb).transpose(1, 0, 2)
            segs.append(seg.reshape(128, DC * tb))
            t0 += tb
        xf = np.ascontiguousarray(np.concatenate(segs, axis=1))

        w1g = np.ascontiguousarray(
            w1[e].reshape(DC, 128, NG1, D_HEAD // NG1).transpose(1, 2, 0, 3)
        ).astype(npdt)  # [128, NG1, DC, 512]
        w2g = np.ascontiguousarray(
            w2[e].reshape(HC, 128, DOC, 128).transpose(1, 2, 0, 3)
        ).astype(npdt)  # [128, DOC, HC, 128]
        b1t = np.ascontiguousarray(b1[e].reshape(HC, 128).T)
        b2t = np.ascontiguousarray(b2[e].reshape(DOC, 128).T)
        gpad = np.zeros(C, np.float32)
        gpad[: len(ie)] = gates[e]
        gb = np.ascontiguousarray(np.broadcast_to(gpad, (128, C)))
        in_maps.append(
            {"xf": xf, "w1g": w1g, "w2g": w2g, "b1t": b1t, "b2t": b2t, "g": gb}
        )
    return in_maps


def kernel(x, gate_w, w1, b1, w2, b2):
    global LAST_RESULT
    x = np.asarray(x, dtype=np.float32)
    B, S, D = x.shape
    x2d = np.ascontiguousarray(x.reshape(-1, D))
    idx, gates = route(x2d, np.asarray(gate_w, np.float32))
    C, blks = plan_blocks(max(len(ie) for ie in idx))
    in_maps = make_in_maps(
        x2d, idx, gates,
        np.asarray(w1, np.float32), np.asarray(b1, np.float32),
        np.asarray(w2, np.float32), np.asarray(b2, np.float32),
        C, blks,
    )
    nc = build_nc(C, blks)
    # run_bass_via_pjrt serializes the module as-is; finalize() runs the
    # Bacc legalization passes (wait splitting, reg alloc) it depends on.
    nc.finalize()
    res = run_bass_kernel_spmd(nc, in_maps, core_ids=list(range(N_CORES)))
    LAST_RESULT = res
    y = np.zeros((B * S, D), np.float64)
    for e in range(N_CORES):
        ie = idx[e]
        y[ie] += res.results[e]["out"][:, : len(ie)].T
    return y.astype(np.float32).reshape(B, S, D)
